# revision 24
# baseline (speedup 1.0000x reference)
"""Single-head causal attention on 8 trn2 NeuronCores.

Problem: x[16, 2048, 1024] fp32, Wq/Wk/Wv[1024, 64] fp32 ->
         out[16, 2048, 64] = softmax(causal(q k^T / sqrt(64))) v

Sharding: data-parallel over batch B=16 -> 2 batches per core, no
collectives. Each core runs an identical (SPMD) Bass program on its own
x shard.

Per-core dataflow (per batch):
  1. DMA x tiles [128, 1024] as f32r, PE-transpose (f32r streams at
     1.5 cyc/row vs 2.0 for f32) into x^T blocks written bf16.
  2. Projections with bf16 weights stationary: [Wq|Wk] packed -> one
     pass gives q^T (partitions 0:64) and k^T (partitions 64:128); k^T
     is partition-shifted to 0:64 by an SBUF->SBUF DMA (bf16, so half
     the bytes). Wv pass gives v^T; small PE transposes give v natural
     [T, 64] with a ones column appended (makes the PV matmul emit the
     softmax denominator for free).
  3. Attention: S^T[Tj part, Ti free] tiles via lhsT=k^T chunk,
     rhs=q^T block (bf16); exp on ACT (scale=1/8 folded; no
     max-subtraction - masked scores max ~13.8, exp fits fp32/bf16)
     writing P^T bf16; causal mask via affine_select on only the
     diagonal 128 columns.
  4. PV in natural orientation: out[Ti,65] += P^T-chunk^T @ v_ext with
     the P^T chunk as the (FWL-fast bf16) stationary and v_ext[s,65]
     streaming -> output lands natural in PSUM, no out-transpose.
     Divide by the l column (col 64), DMA out.
"""

import sys

sys.path.insert(0, "/opt/trn_rl_repo")

import numpy as np

import concourse.bass as bass  # noqa: F401
import concourse.bacc as bacc
import concourse.mybir as mybir
import concourse.tile as tile
from concourse.masks import make_identity
from concourse.bass_utils import run_bass_kernel_spmd

B, T, C, H = 16, 2048, 1024, 64
NCORES = 8
BPC = B // NCORES  # batches per core
CB = C // 128      # 8 contraction chunks
TT = T // 128      # 16 T tiles of 128
NB = T // 512      # 4 T blocks of 512
F32 = mybir.dt.float32
F32R = mybir.dt.float32r
BF16 = mybir.dt.bfloat16
SCALE = float(H) ** -0.5


def build_program(interleave=True, reps=1, tr_bf16_ident=False):
    from contextlib import ExitStack

    nc = bacc.Bacc("TRN2", target_bir_lowering=False, debug=False,
                   num_devices=NCORES)
    # x declared f32r (same bytes as f32) so the DMA'd tiles stream
    # through the PE transpose at f32r rate with no conversion pass.
    x_d = nc.dram_tensor("x", [BPC, T, C], F32R, kind="ExternalInput").ap()
    wq_d = nc.dram_tensor("Wq", [C, H], F32, kind="ExternalInput").ap()
    wk_d = nc.dram_tensor("Wk", [C, H], F32, kind="ExternalInput").ap()
    wv_d = nc.dram_tensor("Wv", [C, H], F32, kind="ExternalInput").ap()
    y_d = nc.dram_tensor("y", [BPC, T, H], F32, kind="ExternalOutput").ap()

    with tile.TileContext(nc) as tc, ExitStack() as ctx:
        singles = ctx.enter_context(tc.tile_pool(name="singles", bufs=1))
        xpool = ctx.enter_context(tc.tile_pool(name="xp", bufs=6))
        xTpool = ctx.enter_context(tc.tile_pool(name="xTp", bufs=3))
        qkpool = ctx.enter_context(tc.tile_pool(name="qkp", bufs=2))
        kTpool = ctx.enter_context(tc.tile_pool(name="kTp", bufs=2))
        vnpool = ctx.enter_context(tc.tile_pool(name="vnp", bufs=2))
        ptpool = ctx.enter_context(tc.tile_pool(name="ptp", bufs=36))
        ypool = ctx.enter_context(tc.tile_pool(name="yp", bufs=4))
        smallp = ctx.enter_context(tc.tile_pool(name="smp", bufs=4))
        ps_tr = ctx.enter_context(tc.tile_pool(name="pstr", bufs=2, space="PSUM"))
        ps_mm = ctx.enter_context(tc.tile_pool(name="psmm", bufs=2, space="PSUM"))
        ps_oa = ctx.enter_context(tc.tile_pool(name="psoa", bufs=2, space="PSUM"))

        ident = singles.tile([128, 128], F32)
        make_identity(nc, ident[:, :])
        ident_r = singles.tile([128, 128], F32R)
        nc.gpsimd.tensor_copy(ident_r[:, :], ident[:, :])
        # full bf16 identity: [0:64,0:64] serves even (base-0) transposes,
        # [64:128,64:128] serves odd (base-64, row-tiled) transposes
        ident_b = singles.tile([128, 128], BF16)
        nc.gpsimd.tensor_copy(ident_b[:, :], ident[:, :])
        # fp32 staging for weights, then a rounding copy to bf16.
        # pass1 packs [Wq | Wv] (q^T at partitions 0:64, v^T at 64:128);
        # pass2 is Wk alone at cols 0:64 so k^T lands at partitions 0:64
        # directly - no partition-shift DMA needed for k.
        wqv_s = singles.tile([128, CB, 128], F32)
        nc.scalar.dma_start(out=wqv_s[:, :, 0:64],
                          in_=wq_d.rearrange("(c p) h -> p c h", p=128))
        nc.scalar.dma_start(out=wqv_s[:, :, 64:128],
                          in_=wv_d.rearrange("(c p) h -> p c h", p=128))
        wk_s = singles.tile([128, CB, 64], F32)
        nc.scalar.dma_start(out=wk_s[:, :, :],
                          in_=wk_d.rearrange("(c p) h -> p c h", p=128))
        wqv = singles.tile([128, CB, 128], BF16)
        wk = singles.tile([128, CB, 64], BF16)
        nc.gpsimd.tensor_copy(wqv[:, :, :], wqv_s[:, :, :])
        nc.gpsimd.tensor_copy(wk[:, :, :], wk_s[:, :, :])
        ones_c = singles.tile([128, 4], BF16)
        nc.gpsimd.memset(ones_c[:, :], 1.0)

        def load_block(b, blk):
            """DMA + PE-transpose one 512-row block of x. Two 128-row
            tiles per DMA (fewer queue-issue slots); the wide PSUM->SBUF
            eviction copies alternate DVE / ACT to split the load."""
            xT = xTpool.tile([128, CB, 512], BF16, tag="xT")
            for t2 in range(2):
                r0 = blk * 512 + t2 * 256
                xt = xpool.tile([128, 2, C], F32R, tag="x")
                nc.sync.dma_start(
                    out=xt[:, :, :],
                    in_=x_d[b, r0:r0 + 256, :].rearrange(
                        "(two p) c -> p two c", p=128))
                for h in range(2):
                    t4 = t2 * 2 + h
                    # 8 transposes fill one 2-bank PSUM tile -> single
                    # wide copy (each matmul output stays in one bank)
                    tp8 = ps_tr.tile([128, 1024], F32R, tag="tr")
                    for ci in range(CB):
                        nc.tensor.matmul(tp8[:, ci * 128:(ci + 1) * 128],
                                         xt[:, h, ci * 128:(ci + 1) * 128],
                                         ident_r[:, :], is_transpose=True)
                    dst = xT[:, :, t4 * 128:(t4 + 1) * 128]
                    s8 = tp8[:, :].rearrange("p (c t) -> p c t", c=CB)
                    if t4 % 2 == 0:
                        nc.vector.tensor_copy(dst, s8)
                    else:
                        nc.scalar.activation(
                            dst, s8, mybir.ActivationFunctionType.Copy)
            return xT

        def phase_ab(b, st):
            """Load + transpose x, projections. Yields once per T block."""
            # one tile holds q^T (rows 0:64) and v^T (rows 64:128) so the
            # projection PSUM evicts in a single wide copy
            qvT = qkpool.tile([128, T], BF16, tag="qvT")
            kT = kTpool.tile([64, T], BF16, tag="kT")
            vn = vnpool.tile([128, TT, 65], BF16, tag="vn")
            st["qT"], st["kT"], st["vn"] = qvT, kT, vn
            def proj(blk, xT):
                sl = slice(blk * 512, (blk + 1) * 512)
                pq = ps_mm.tile([128, 512], F32, tag="mm")
                for ci in range(CB):
                    nc.tensor.matmul(pq[:, :], wqv[:, ci, :], xT[:, ci, :],
                                     start=(ci == 0), stop=(ci == CB - 1))
                nc.vector.tensor_copy(qvT[:, sl], pq[:, :])
                pk = ps_mm.tile([64, 512], F32, tag="mm")
                for ci in range(CB):
                    nc.tensor.matmul(pk[:, :], wk[:, ci, :], xT[:, ci, :],
                                     start=(ci == 0), stop=(ci == CB - 1))
                nc.vector.tensor_copy(kT[:, sl], pk[:, :])
                # v natural [Tj, 64] tiles from the base-64 v^T rows: 4
                # transposes share one PSUM bank -> one wide copy
                tpv = ps_tr.tile([128, 256], BF16, tag="tr")
                for t4 in range(4):
                    tj = blk * 4 + t4
                    nc.tensor.matmul(tpv[:, t4 * 64:(t4 + 1) * 64],
                                     qvT[64:128, tj * 128:(tj + 1) * 128],
                                     ident_b[64:128, 64:128],
                                     is_transpose=True)
                nc.vector.tensor_copy(
                    vn[:, blk * 4:blk * 4 + 4, 0:64],
                    tpv[:, :].rearrange("p (c h) -> p c h", c=4))
                nc.gpsimd.tensor_copy(vn[:, blk * 4:blk * 4 + 4, 64],
                                      ones_c[:, :])

            prev = None
            for blk in range(NB):
                xT = load_block(b, blk)
                if prev is not None:
                    proj(*prev)
                prev = (blk, xT)
                if blk == NB - 1:
                    proj(*prev)
                yield

        def phase_c(b, st, order=None):
            """Attention. Yields once per Ti block of 512 queries. The PV
            stage lags scores by one block so the in-order PE queue always
            has score matmuls to run while ACT exps the previous block."""
            qT, kT, vn = st["qT"], st["kT"], st["vn"]

            def scores(bi):
                pts = []
                for j in range(4 * bi + 4):
                    r = j - 4 * bi
                    if r <= 0:
                        w, c0 = 512, 0
                    else:
                        w, c0 = 512 - 128 * r, 128 * r
                    stt = ps_mm.tile([128, w], F32, tag="mm")
                    nc.tensor.matmul(
                        stt[:, :], kT[:, j * 128:(j + 1) * 128],
                        qT[0:64, bi * 512 + c0:(bi + 1) * 512],
                        start=True, stop=True)
                    pt = ptpool.tile([128, w], BF16, tag="pt")
                    nc.scalar.activation(pt[:, :], stt[:, :],
                                         mybir.ActivationFunctionType.Exp,
                                         scale=SCALE)
                    if r >= 0:
                        # diagonal chunk: keep (within-tile free idx) >=
                        # partition idx; only the first 128 cols straddle
                        # the diagonal, the rest are strictly below it
                        nc.gpsimd.affine_select(
                            out=pt[:, 0:128], in_=pt[:, 0:128],
                            compare_op=mybir.AluOpType.is_ge, fill=0.0,
                            base=0, pattern=[[1, 128]], channel_multiplier=-1)
                    pts.append((pt, c0))
                return pts

            def pv(bi, pts):
                # PV natural: P^T chunk stationary, v_ext streaming;
                # out[Ti, 65] accumulates in PSUM, no out-transpose
                yt4 = ypool.tile([128, 4, 64], F32, tag="yt")
                for i4 in range(4):
                    oacc = ps_oa.tile([128, 65], F32, tag="oa")
                    for j in range(4 * bi + i4 + 1):
                        pt, c0 = pts[j]
                        cs = i4 * 128 - c0
                        nc.tensor.matmul(oacc[:, :],
                                         pt[:, cs:cs + 128], vn[:, j, :],
                                         start=(j == 0),
                                         stop=(j == 4 * bi + i4))
                    linv = smallp.tile([128, 1], F32, tag="linv")
                    nc.vector.reciprocal(linv[:, :], oacc[:, 64:65])
                    nc.vector.tensor_scalar_mul(yt4[:, i4, :], oacc[:, 0:64],
                                                linv[:, :])
                # one DMA stores the whole 512-row block
                nc.sync.dma_start(
                    out=y_d[b, bi * 512:(bi + 1) * 512, :].rearrange(
                        "(i4 p) h -> p i4 h", p=128),
                    in_=yt4[:, :, :])

            blocks = list(order or range(NB))
            prev = None
            for n, bi in enumerate(blocks):
                pts = scores(bi)
                if prev is not None:
                    pv(*prev)
                prev = (bi, pts)
                if n == NB - 1:
                    pv(*prev)
                yield

        def drain(g):
            for _ in g:
                pass

        def body():
            states = [dict() for _ in range(BPC)]
            if not interleave or BPC == 1:
                for b in range(BPC):
                    drain(phase_ab(b, states[b]))
                    drain(phase_c(b, states[b]))
                return
            # Fine-grained software pipeline. C(b) block bi only needs
            # AB(b) blocks 0..bi, so attention starts one block after the
            # projections and trails them by one block throughout; only
            # C(1)'s last block runs without overlap.
            ab0, c0 = phase_ab(0, states[0]), phase_c(0, states[0])
            ab1 = phase_ab(1, states[1])
            c1 = phase_c(1, states[1])
            next(ab0)                      # AB0.b0
            for _ in range(NB - 1):        # AB0.b1-3 | C0.b0-2
                next(ab0)
                next(c0)
            next(ab1)                      # AB1.b0 | C0.b3
            next(c0)
            for _ in range(NB - 1):        # AB1.b1-3 | C1.b0-2
                next(ab1)
                next(c1)
            next(c1)                       # C1.b3
            for g in (ab0, c0, ab1, c1):
                drain(g)

        if reps == 1:
            body()
        else:
            with tc.For_i(0, reps, 1):
                body()

    nc.compile()
    return nc


_CACHE = {}


def _get_program(**kw):
    key = tuple(sorted(kw.items()))
    if key not in _CACHE:
        _CACHE[key] = build_program(**kw)
    return _CACHE[key]


def run_sharded(x, Wq, Wk, Wv, trace=False, **build_kw):
    """Run on 8 cores, return (y_full, BassKernelResults)."""
    nc = _get_program(**build_kw)
    x = np.ascontiguousarray(np.asarray(x, dtype=np.float32))
    Wq = np.ascontiguousarray(np.asarray(Wq, dtype=np.float32))
    Wk = np.ascontiguousarray(np.asarray(Wk, dtype=np.float32))
    Wv = np.ascontiguousarray(np.asarray(Wv, dtype=np.float32))
    xs = x.reshape(NCORES, BPC, T, C)
    in_maps = [{"x": np.ascontiguousarray(xs[i]), "Wq": Wq, "Wk": Wk, "Wv": Wv}
               for i in range(NCORES)]
    res = run_bass_kernel_spmd(nc, in_maps, list(range(NCORES)), trace=trace)
    y = np.stack([res.results[i]["y"] for i in range(NCORES)], axis=0)
    return y.reshape(B, T, H), res


def kernel(x, Wq, Wk, Wv):
    y, _ = run_sharded(x, Wq, Wk, Wv, trace=False)
    return y


# ---------------- timing support (no NTFF profiler in this container) ----


def make_runner(nc, n_iter=1):
    """Build a reusable sharded jit callable for `nc` (mirrors
    bass2jax.run_bass_via_pjrt's multi-core path, without donation so
    device inputs can be reused across timed calls). n_iter > 1 chains
    the NEFF invocation serially (output buffers fed back as the next
    call's output-operands) so per-invocation time can be measured as a
    slope, independent of the ~90 ms axon dispatch floor."""
    import jax
    from jax.sharding import Mesh, PartitionSpec
    try:
        from jax.experimental.shard_map import shard_map
    except ImportError:  # newer jax
        from jax.shard_map import shard_map
    from concourse import bass2jax
    bass2jax.install_neuronx_cc_hook()

    part_name = (nc.partition_id_tensor.name if nc.partition_id_tensor
                 else None)
    in_names, out_names, out_avals, zero_outs = [], [], [], []
    for alloc in nc.m.functions[0].allocations:
        if not isinstance(alloc, mybir.MemoryLocationSet):
            continue
        name = alloc.memorylocations[0].name
        if alloc.kind == "ExternalInput":
            if name != part_name:
                in_names.append(name)
        elif alloc.kind == "ExternalOutput":
            out_names.append(name)
            shape = tuple(alloc.tensor_shape)
            dtype = mybir.dt.np(alloc.dtype)
            out_avals.append(jax.core.ShapedArray(shape, dtype))
            zero_outs.append(np.zeros(shape, dtype))
    n_params = len(in_names)
    all_names = in_names + out_names
    if part_name is not None:
        all_names = all_names + [part_name]

    def _body(*args):
        ins = list(args[:n_params])
        youts = list(args[n_params:n_params + len(out_names)])
        for _ in range(n_iter):
            operands = ins + youts
            if part_name is not None:
                operands.append(bass2jax.partition_id_tensor())
            outs = bass2jax._bass_exec_p.bind(
                *operands, out_avals=tuple(out_avals),
                in_names=tuple(all_names), out_names=tuple(out_names),
                lowering_input_output_aliases=(),
                sim_require_finite=True, sim_require_nnan=True, nc=nc)
            youts = list(outs)
        return tuple(youts)

    devices = jax.devices()[:NCORES]
    mesh = Mesh(np.asarray(devices), ("core",))
    in_specs = (PartitionSpec("core"),) * (n_params + len(out_names))
    out_specs = (PartitionSpec("core"),) * len(out_names)
    fn = jax.jit(shard_map(_body, mesh=mesh, in_specs=in_specs,
                           out_specs=out_specs, check_rep=False),
                 keep_unused=True)
    return fn, in_names, zero_outs, mesh


def _timed_calls(fn, dev_in, iters):
    import time as _time
    import jax
    out = fn(*dev_in)
    jax.block_until_ready(out)
    ts = []
    for _ in range(iters):
        t0 = _time.perf_counter_ns()
        out = fn(*dev_in)
        jax.block_until_ready(out)
        ts.append(_time.perf_counter_ns() - t0)
    ts.sort()
    return ts


def time_calls(nc, in_maps, iters=10):
    """Sorted wall times (ns) of warm sharded calls of nc's NEFF."""
    import jax
    from jax.sharding import NamedSharding, PartitionSpec
    fn, in_names, zero_outs, mesh = make_runner(nc, n_iter=1)
    sh = NamedSharding(mesh, PartitionSpec("core"))
    concat = [np.concatenate([np.asarray(m[n]) for m in in_maps], axis=0)
              for n in in_names]
    concat += [np.zeros((NCORES * z.shape[0], *z.shape[1:]), z.dtype)
               for z in zero_outs]
    dev_in = [jax.device_put(a, sh) for a in concat]
    return _timed_calls(fn, dev_in, iters)


# revision 25
# speedup vs baseline: 1.2923x; 1.2923x over previous
"""Single-head causal attention on 8 trn2 NeuronCores.

Problem: x[16, 2048, 1024] fp32, Wq/Wk/Wv[1024, 64] fp32 ->
         out[16, 2048, 64] = softmax(causal(q k^T / sqrt(64))) v

Sharding: data-parallel over batch B=16 -> 2 batches per core, no
collectives. Each core runs an identical (SPMD) Bass program on its own
x shard.

Per-core dataflow (per batch):
  1. DMA x tiles [128, 1024] as f32r, PE-transpose (f32r streams at
     1.5 cyc/row vs 2.0 for f32) into x^T blocks written bf16.
  2. Projections with bf16 weights stationary: [Wq|Wk] packed -> one
     pass gives q^T (partitions 0:64) and k^T (partitions 64:128); k^T
     is partition-shifted to 0:64 by an SBUF->SBUF DMA (bf16, so half
     the bytes). Wv pass gives v^T; small PE transposes give v natural
     [T, 64] with a ones column appended (makes the PV matmul emit the
     softmax denominator for free).
  3. Attention: S^T[Tj part, Ti free] tiles via lhsT=k^T chunk,
     rhs=q^T block (bf16); exp on ACT (scale=1/8 folded; no
     max-subtraction - masked scores max ~13.8, exp fits fp32/bf16)
     writing P^T bf16; causal mask via affine_select on only the
     diagonal 128 columns.
  4. PV in natural orientation: out[Ti,65] += P^T-chunk^T @ v_ext with
     the P^T chunk as the (FWL-fast bf16) stationary and v_ext[s,65]
     streaming -> output lands natural in PSUM, no out-transpose.
     Divide by the l column (col 64), DMA out.
"""

import sys

sys.path.insert(0, "/opt/trn_rl_repo")

import numpy as np

import concourse.bass as bass  # noqa: F401
import concourse.bacc as bacc
import concourse.mybir as mybir
import concourse.tile as tile
from concourse.masks import make_identity
from concourse.bass_utils import run_bass_kernel_spmd

B, T, C, H = 16, 2048, 1024, 64
NCORES = 8
BPC = B // NCORES  # batches per core
CB = C // 128      # 8 contraction chunks
TT = T // 128      # 16 T tiles of 128
NB = T // 512      # 4 T blocks of 512
F32 = mybir.dt.float32
F32R = mybir.dt.float32r
BF16 = mybir.dt.bfloat16
SCALE = float(H) ** -0.5


def build_program(interleave=True, reps=1, tr_bf16_ident=False):
    from contextlib import ExitStack

    nc = bacc.Bacc("TRN2", target_bir_lowering=False, debug=False,
                   num_devices=NCORES)
    # x declared f32r (same bytes as f32) so the DMA'd tiles stream
    # through the PE transpose at f32r rate with no conversion pass.
    x_d = nc.dram_tensor("x", [BPC, T, C], F32R, kind="ExternalInput").ap()
    wq_d = nc.dram_tensor("Wq", [C, H], F32, kind="ExternalInput").ap()
    wk_d = nc.dram_tensor("Wk", [C, H], F32, kind="ExternalInput").ap()
    wv_d = nc.dram_tensor("Wv", [C, H], F32, kind="ExternalInput").ap()
    y_d = nc.dram_tensor("y", [BPC, T, H], F32, kind="ExternalOutput").ap()

    with tile.TileContext(nc) as tc, ExitStack() as ctx:
        singles = ctx.enter_context(tc.tile_pool(name="singles", bufs=1))
        xpool = ctx.enter_context(tc.tile_pool(name="xp", bufs=6))
        xTpool = ctx.enter_context(tc.tile_pool(name="xTp", bufs=3))
        qkpool = ctx.enter_context(tc.tile_pool(name="qkp", bufs=2))
        kTpool = ctx.enter_context(tc.tile_pool(name="kTp", bufs=2))
        vnpool = ctx.enter_context(tc.tile_pool(name="vnp", bufs=2))
        ptpool = ctx.enter_context(tc.tile_pool(name="ptp", bufs=36))
        ypool = ctx.enter_context(tc.tile_pool(name="yp", bufs=4))
        smallp = ctx.enter_context(tc.tile_pool(name="smp", bufs=4))
        ps_tr = ctx.enter_context(tc.tile_pool(name="pstr", bufs=2, space="PSUM"))
        ps_mm = ctx.enter_context(tc.tile_pool(name="psmm", bufs=2, space="PSUM"))
        ps_oa = ctx.enter_context(tc.tile_pool(name="psoa", bufs=2, space="PSUM"))

        ident = singles.tile([128, 128], F32)
        make_identity(nc, ident[:, :])
        ident_r = singles.tile([128, 128], F32R)
        nc.gpsimd.tensor_copy(ident_r[:, :], ident[:, :])
        # full bf16 identity: [0:64,0:64] serves even (base-0) transposes,
        # [64:128,64:128] serves odd (base-64, row-tiled) transposes
        ident_b = singles.tile([128, 128], BF16)
        nc.gpsimd.tensor_copy(ident_b[:, :], ident[:, :])
        # fp32 staging for weights, then a rounding copy to bf16.
        # pass1 packs [Wq | Wv] (q^T at partitions 0:64, v^T at 64:128);
        # pass2 is Wk alone at cols 0:64 so k^T lands at partitions 0:64
        # directly - no partition-shift DMA needed for k.
        wqv_s = singles.tile([128, CB, 128], F32)
        nc.scalar.dma_start(out=wqv_s[:, :, 0:64],
                          in_=wq_d.rearrange("(c p) h -> p c h", p=128))
        nc.scalar.dma_start(out=wqv_s[:, :, 64:128],
                          in_=wv_d.rearrange("(c p) h -> p c h", p=128))
        wk_s = singles.tile([128, CB, 64], F32)
        nc.scalar.dma_start(out=wk_s[:, :, :],
                          in_=wk_d.rearrange("(c p) h -> p c h", p=128))
        wqv = singles.tile([128, CB, 128], BF16)
        wk = singles.tile([128, CB, 64], BF16)
        nc.gpsimd.tensor_copy(wqv[:, :, :], wqv_s[:, :, :])
        nc.gpsimd.tensor_copy(wk[:, :, :], wk_s[:, :, :])
        ones_c = singles.tile([128, 4], BF16)
        nc.gpsimd.memset(ones_c[:, :], 1.0)

        def issue_loads(b, blk):
            """Issue the two x DMAs for one 512-row block (prefetch)."""
            xts = []
            for t2 in range(2):
                r0 = blk * 512 + t2 * 256
                xt = xpool.tile([128, 2, C], F32R, tag="x")
                nc.sync.dma_start(
                    out=xt[:, :, :],
                    in_=x_d[b, r0:r0 + 256, :].rearrange(
                        "(two p) c -> p two c", p=128))
                xts.append(xt)
            return xts

        def transpose_block(xts):
            """PE-transpose one block of prefetched x tiles. The wide
            PSUM->SBUF eviction copies alternate DVE / ACT."""
            xT = xTpool.tile([128, CB, 512], BF16, tag="xT")
            for t4 in range(4):
                xt, h = xts[t4 // 2], t4 % 2
                # 8 transposes fill one 2-bank PSUM tile -> single
                # wide copy (each matmul output stays in one bank)
                tp8 = ps_tr.tile([128, 1024], F32R, tag="tr")
                for ci in range(CB):
                    nc.tensor.matmul(tp8[:, ci * 128:(ci + 1) * 128],
                                     xt[:, h, ci * 128:(ci + 1) * 128],
                                     ident_r[:, :], is_transpose=True)
                dst = xT[:, :, t4 * 128:(t4 + 1) * 128]
                s8 = tp8[:, :].rearrange("p (c t) -> p c t", c=CB)
                if t4 % 2 == 0:
                    nc.vector.tensor_copy(dst, s8)
                else:
                    nc.scalar.activation(
                        dst, s8, mybir.ActivationFunctionType.Copy)
            return xT

        def phase_ab(b, st):
            """Load + transpose x, projections. Yields once per T block."""
            # one tile holds q^T (rows 0:64) and v^T (rows 64:128) so the
            # projection PSUM evicts in a single wide copy
            qvT = qkpool.tile([128, T], BF16, tag="qvT")
            kT = kTpool.tile([64, T], BF16, tag="kT")
            vn = vnpool.tile([128, TT, 65], BF16, tag="vn")
            st["qT"], st["kT"], st["vn"] = qvT, kT, vn
            def proj(blk, xT):
                sl = slice(blk * 512, (blk + 1) * 512)
                pq = ps_mm.tile([128, 512], F32, tag="mm")
                for ci in range(CB):
                    nc.tensor.matmul(pq[:, :], wqv[:, ci, :], xT[:, ci, :],
                                     start=(ci == 0), stop=(ci == CB - 1))
                nc.vector.tensor_copy(qvT[:, sl], pq[:, :])
                pk = ps_mm.tile([64, 512], F32, tag="mm")
                for ci in range(CB):
                    nc.tensor.matmul(pk[:, :], wk[:, ci, :], xT[:, ci, :],
                                     start=(ci == 0), stop=(ci == CB - 1))
                nc.vector.tensor_copy(kT[:, sl], pk[:, :])
                # v natural [Tj, 64] tiles from the base-64 v^T rows: 4
                # transposes share one PSUM bank -> one wide copy
                tpv = ps_tr.tile([128, 256], BF16, tag="tr")
                for t4 in range(4):
                    tj = blk * 4 + t4
                    nc.tensor.matmul(tpv[:, t4 * 64:(t4 + 1) * 64],
                                     qvT[64:128, tj * 128:(tj + 1) * 128],
                                     ident_b[64:128, 64:128],
                                     is_transpose=True)
                nc.vector.tensor_copy(
                    vn[:, blk * 4:blk * 4 + 4, 0:64],
                    tpv[:, :].rearrange("p (c h) -> p c h", c=4))
                nc.gpsimd.tensor_copy(vn[:, blk * 4:blk * 4 + 4, 64],
                                      ones_c[:, :])

            prev = None
            pend = issue_loads(b, 0)
            for blk in range(NB):
                # prefetch next block's x DMAs before this block's compute
                nxt = issue_loads(b, blk + 1) if blk + 1 < NB else None
                xT = transpose_block(pend)
                pend = nxt
                if prev is not None:
                    proj(*prev)
                prev = (blk, xT)
                if blk == NB - 1:
                    proj(*prev)
                yield

        def phase_c(b, st, order=None):
            """Attention. Yields once per Ti block of 512 queries. The PV
            stage lags scores by one block so the in-order PE queue always
            has score matmuls to run while ACT exps the previous block."""
            qT, kT, vn = st["qT"], st["kT"], st["vn"]

            def scores(bi):
                pts = []
                for j in range(4 * bi + 4):
                    r = j - 4 * bi
                    if r <= 0:
                        w, c0 = 512, 0
                    else:
                        w, c0 = 512 - 128 * r, 128 * r
                    stt = ps_mm.tile([128, w], F32, tag="mm")
                    nc.tensor.matmul(
                        stt[:, :], kT[:, j * 128:(j + 1) * 128],
                        qT[0:64, bi * 512 + c0:(bi + 1) * 512],
                        start=True, stop=True)
                    pt = ptpool.tile([128, w], BF16, tag="pt")
                    nc.scalar.activation(pt[:, :], stt[:, :],
                                         mybir.ActivationFunctionType.Exp,
                                         scale=SCALE)
                    if r >= 0:
                        # diagonal chunk: keep (within-tile free idx) >=
                        # partition idx; only the first 128 cols straddle
                        # the diagonal, the rest are strictly below it
                        nc.gpsimd.affine_select(
                            out=pt[:, 0:128], in_=pt[:, 0:128],
                            compare_op=mybir.AluOpType.is_ge, fill=0.0,
                            base=0, pattern=[[1, 128]], channel_multiplier=-1)
                    pts.append((pt, c0))
                return pts

            def pv(bi, pts):
                # PV natural: P^T chunk stationary, v_ext streaming;
                # out[Ti, 65] accumulates in PSUM, no out-transpose
                yt4 = ypool.tile([128, 4, 64], F32, tag="yt")
                for i4 in range(4):
                    oacc = ps_oa.tile([128, 65], F32, tag="oa")
                    for j in range(4 * bi + i4 + 1):
                        pt, c0 = pts[j]
                        cs = i4 * 128 - c0
                        nc.tensor.matmul(oacc[:, :],
                                         pt[:, cs:cs + 128], vn[:, j, :],
                                         start=(j == 0),
                                         stop=(j == 4 * bi + i4))
                    linv = smallp.tile([128, 1], F32, tag="linv")
                    nc.vector.reciprocal(linv[:, :], oacc[:, 64:65])
                    nc.vector.tensor_scalar_mul(yt4[:, i4, :], oacc[:, 0:64],
                                                linv[:, :])
                # one DMA stores the whole 512-row block
                nc.sync.dma_start(
                    out=y_d[b, bi * 512:(bi + 1) * 512, :].rearrange(
                        "(i4 p) h -> p i4 h", p=128),
                    in_=yt4[:, :, :])

            blocks = list(order or range(NB))
            prev = None
            for n, bi in enumerate(blocks):
                pts = scores(bi)
                if prev is not None:
                    pv(*prev)
                prev = (bi, pts)
                if n == NB - 1:
                    pv(*prev)
                yield

        def drain(g):
            for _ in g:
                pass

        def body():
            states = [dict() for _ in range(BPC)]
            if not interleave or BPC == 1:
                for b in range(BPC):
                    drain(phase_ab(b, states[b]))
                    drain(phase_c(b, states[b]))
                return
            # Fine-grained software pipeline. C(b) block bi only needs
            # AB(b) blocks 0..bi, so attention starts one block after the
            # projections and trails them by one block throughout; only
            # C(1)'s last block runs without overlap.
            ab0, c0 = phase_ab(0, states[0]), phase_c(0, states[0])
            ab1 = phase_ab(1, states[1])
            c1 = phase_c(1, states[1])
            next(ab0)                      # AB0.b0
            for _ in range(NB - 1):        # AB0.b1-3 | C0.b0-2
                next(ab0)
                next(c0)
            next(ab1)                      # AB1.b0 | C0.b3
            next(c0)
            for _ in range(NB - 1):        # AB1.b1-3 | C1.b0-2
                next(ab1)
                next(c1)
            next(c1)                       # C1.b3
            for g in (ab0, c0, ab1, c1):
                drain(g)

        if reps == 1:
            body()
        else:
            with tc.For_i(0, reps, 1):
                body()

    nc.compile()
    return nc


_CACHE = {}


def _get_program(**kw):
    key = tuple(sorted(kw.items()))
    if key not in _CACHE:
        _CACHE[key] = build_program(**kw)
    return _CACHE[key]


def run_sharded(x, Wq, Wk, Wv, trace=False, **build_kw):
    """Run on 8 cores, return (y_full, BassKernelResults)."""
    nc = _get_program(**build_kw)
    x = np.ascontiguousarray(np.asarray(x, dtype=np.float32))
    Wq = np.ascontiguousarray(np.asarray(Wq, dtype=np.float32))
    Wk = np.ascontiguousarray(np.asarray(Wk, dtype=np.float32))
    Wv = np.ascontiguousarray(np.asarray(Wv, dtype=np.float32))
    xs = x.reshape(NCORES, BPC, T, C)
    in_maps = [{"x": np.ascontiguousarray(xs[i]), "Wq": Wq, "Wk": Wk, "Wv": Wv}
               for i in range(NCORES)]
    res = run_bass_kernel_spmd(nc, in_maps, list(range(NCORES)), trace=trace)
    y = np.stack([res.results[i]["y"] for i in range(NCORES)], axis=0)
    return y.reshape(B, T, H), res


def kernel(x, Wq, Wk, Wv):
    y, _ = run_sharded(x, Wq, Wk, Wv, trace=False)
    return y


# ---------------- timing support (no NTFF profiler in this container) ----


def make_runner(nc, n_iter=1):
    """Build a reusable sharded jit callable for `nc` (mirrors
    bass2jax.run_bass_via_pjrt's multi-core path, without donation so
    device inputs can be reused across timed calls). n_iter > 1 chains
    the NEFF invocation serially (output buffers fed back as the next
    call's output-operands) so per-invocation time can be measured as a
    slope, independent of the ~90 ms axon dispatch floor."""
    import jax
    from jax.sharding import Mesh, PartitionSpec
    try:
        from jax.experimental.shard_map import shard_map
    except ImportError:  # newer jax
        from jax.shard_map import shard_map
    from concourse import bass2jax
    bass2jax.install_neuronx_cc_hook()

    part_name = (nc.partition_id_tensor.name if nc.partition_id_tensor
                 else None)
    in_names, out_names, out_avals, zero_outs = [], [], [], []
    for alloc in nc.m.functions[0].allocations:
        if not isinstance(alloc, mybir.MemoryLocationSet):
            continue
        name = alloc.memorylocations[0].name
        if alloc.kind == "ExternalInput":
            if name != part_name:
                in_names.append(name)
        elif alloc.kind == "ExternalOutput":
            out_names.append(name)
            shape = tuple(alloc.tensor_shape)
            dtype = mybir.dt.np(alloc.dtype)
            out_avals.append(jax.core.ShapedArray(shape, dtype))
            zero_outs.append(np.zeros(shape, dtype))
    n_params = len(in_names)
    all_names = in_names + out_names
    if part_name is not None:
        all_names = all_names + [part_name]

    def _body(*args):
        ins = list(args[:n_params])
        youts = list(args[n_params:n_params + len(out_names)])
        for _ in range(n_iter):
            operands = ins + youts
            if part_name is not None:
                operands.append(bass2jax.partition_id_tensor())
            outs = bass2jax._bass_exec_p.bind(
                *operands, out_avals=tuple(out_avals),
                in_names=tuple(all_names), out_names=tuple(out_names),
                lowering_input_output_aliases=(),
                sim_require_finite=True, sim_require_nnan=True, nc=nc)
            youts = list(outs)
        return tuple(youts)

    devices = jax.devices()[:NCORES]
    mesh = Mesh(np.asarray(devices), ("core",))
    in_specs = (PartitionSpec("core"),) * (n_params + len(out_names))
    out_specs = (PartitionSpec("core"),) * len(out_names)
    fn = jax.jit(shard_map(_body, mesh=mesh, in_specs=in_specs,
                           out_specs=out_specs, check_rep=False),
                 keep_unused=True)
    return fn, in_names, zero_outs, mesh


def _timed_calls(fn, dev_in, iters):
    import time as _time
    import jax
    out = fn(*dev_in)
    jax.block_until_ready(out)
    ts = []
    for _ in range(iters):
        t0 = _time.perf_counter_ns()
        out = fn(*dev_in)
        jax.block_until_ready(out)
        ts.append(_time.perf_counter_ns() - t0)
    ts.sort()
    return ts


def time_calls(nc, in_maps, iters=10):
    """Sorted wall times (ns) of warm sharded calls of nc's NEFF."""
    import jax
    from jax.sharding import NamedSharding, PartitionSpec
    fn, in_names, zero_outs, mesh = make_runner(nc, n_iter=1)
    sh = NamedSharding(mesh, PartitionSpec("core"))
    concat = [np.concatenate([np.asarray(m[n]) for m in in_maps], axis=0)
              for n in in_names]
    concat += [np.zeros((NCORES * z.shape[0], *z.shape[1:]), z.dtype)
               for z in zero_outs]
    dev_in = [jax.device_put(a, sh) for a in concat]
    return _timed_calls(fn, dev_in, iters)
